# revision 69
# baseline (speedup 1.0000x reference)
"""MoE multi-head attention Trainium2 kernel (v4, fused router columns).

Problem: x:[B=2,S=2048,D=1024], Wq:[H=4,E=4,D,DH=256], Wk/Wv:[D,D], Wr:[H,E*DH,E]
  K/V = per-head projections of x; Q per (head, expert); full softmax attention
  per (b,h,e); router softmax over experts from concat of expert outputs;
  router-weighted combine -> out [B,S,H,DH].

Sharding: 8 cores = B*H (2 batches x 4 heads); all E=4 experts core-local.

Large GEMMs run as fp8e4 DoubleRow matmuls (2 contraction tiles per pass,
0.5 cyc/row) with hi/lo error compensation (3-chain ah*bh + ah*bl + al*bh).
Attention weights `at` stay bf16: an fp8 `at` (or any dropped chain) was
measured at 2-4e-2 final error vs the 2e-2 gate, so eo runs bf16 at the
bf16 roofline.

v4 change vs v3: the router/rowsum path no longer uses 1024 separate tiny
matmuls (the PE *sequencer* costs ~75ns/matmul — those "free" ap=5 matmuls
were 77us of SEQ time). Instead each expert's V tile carries 5 fused
columns [ones | U_e(4)] (U = Wv@Wr precomputed on host), so the eo
accumulation chain computes eo, the softmax rowsum, AND the per-expert
router logits in a single ap=261 matmul per (t,chunk).
PSUM (8 banks): scores 3x1 + eoA/eoB 2x2 (chunk c of a pair-tile at col
512c, own bank each) + a dedicated Q-projection bank.

Scheduling notes (each worth ~0.5-3us of PE idle, found via TimelineSim
traces):
 - one large DMA per s-tile of x, two queues alternating, in exact
   consumption order (HWDGE descriptor gen ~0.6us serializes transfers
   across queues; dispatch costs ~0.7-1.3us of sequencer per DMA);
   a DMA must be EMITTED before its first reader or the tile framework
   orders it as WAR after the read.
 - sc/exp per single t-tile, eo consumers LAG=5 tiles behind.
 - Q-projection for unit u+2 emitted inside unit u's batch loop; its fp8
   quantize runs fully on DVE in phase 2 (an ACT copy would land in the
   boundary window where ACT must drain the LAG-tail exps that free scp
   slots) but on ACT in phase 1 (there DVE is the congested engine).
 - each expert's eoB drain is deferred into the NEXT expert's batch loop
   (same reason); eoA drain + reciprocals + logit-accumulate stay at the
   boundary, smallest ops first.
 - the previous tile's 16-op DVE combine is emitted one expert-layer at a
   time at the next tile's expert boundaries.
 - last tile: combine runs on the then-idle PE as diag(w_e) @ eo_e PSUM
   accumulation chains (identity row-scaled by DVE), pipelined per chunk
   behind a per-chunk router-softmax chain, with per-chunk stores.
"""
import sys

sys.path.insert(0, "/opt/trn_rl_repo")

import math

import numpy as np
import ml_dtypes

import concourse.bass as bass
import concourse.mybir as mybir
import concourse.tile as tile
from concourse import bacc, bass_utils

B, S, D = 2, 2048, 1024
H, E, DH = 4, 4, 256
SCALE = math.sqrt(DH)
NCORES = B * H

DC = D // 128      # 8 contraction chunks over D
DP = DC // 2       # 4 DoubleRow chunk-pairs over D
KC = DH // 128     # 2 chunks over head dim
ST = S // 512      # 4 tiles of 512 queries
TT = S // 128      # 16 tiles of 128 tokens
NCH = 4            # 128-query chunks per s-tile
VW = DH + 1 + E    # per-(tt,e) va block: [V(256) | ones | U_e(4)]
LN4 = math.log(4.0)

SX = 16.0          # fp8 scale for x, K, Q values (|v| ~ N(0,1))
SW = 512.0         # fp8 scale for weights (|w| ~ N(0, 1/1024))

F32 = mybir.dt.float32
BF16 = mybir.dt.bfloat16
FP8 = mybir.dt.float8e4
DR = mybir.MatmulPerfMode.DoubleRow

_cached = None
_last_in_maps = None


def _build(upto=3):
    nc = bacc.Bacc("TRN2", target_bir_lowering=False, debug=False)

    # hi and lo halves in ONE tensor: one DMA per s-tile (HWDGE descriptor
    # generation is ~0.6us per DMA and serializes across queues)
    x8_d = nc.dram_tensor("x8", [128, 2 * DC * S], FP8, kind="ExternalInput")
    wk8h_d = nc.dram_tensor("wk8h", [128, DC * DH], FP8, kind="ExternalInput")
    wk8l_d = nc.dram_tensor("wk8l", [128, DC * DH], FP8, kind="ExternalInput")
    wv8h_d = nc.dram_tensor("wv8h", [128, DC * DH], FP8, kind="ExternalInput")
    wv8l_d = nc.dram_tensor("wv8l", [128, DC * DH], FP8, kind="ExternalInput")
    wq8h_d = nc.dram_tensor("wq8h", [128, E * DC * DH], FP8, kind="ExternalInput")
    wq8l_d = nc.dram_tensor("wq8l", [128, E * DC * DH], FP8, kind="ExternalInput")
    w28h_d = nc.dram_tensor("w28h", [128, DC * E * E], FP8, kind="ExternalInput")
    biasc_d = nc.dram_tensor("biasc", [128, 1], F32, kind="ExternalInput")
    zeroc_d = nc.dram_tensor("zeroc", [128, 1], F32, kind="ExternalInput")
    ident_d = nc.dram_tensor("ident", [128, 128], BF16, kind="ExternalInput")
    out_d = nc.dram_tensor("out", [S, DH], BF16, kind="ExternalOutput")
    if upto == 1:
        dbg_k = nc.dram_tensor("dbg_k", [128, KC * S], FP8, kind="ExternalOutput")
        dbg_kl = nc.dram_tensor("dbg_kl", [128, KC * S], FP8, kind="ExternalOutput")
        dbg_v = nc.dram_tensor("dbg_v", [128, TT * E * VW], BF16, kind="ExternalOutput")

    with tile.TileContext(nc) as tc:
        with (
            tc.tile_pool(name="pw", bufs=1) as pw,
            tc.tile_pool(name="pkv", bufs=1) as pkv,
        ):
            # ---- resident weights/constants ----
            wk8h_sb = pw.tile([128, DC * DH], FP8)
            wk8l_sb = pw.tile([128, DC * DH], FP8)
            wv8h_sb = pw.tile([128, DC * DH], FP8)
            wv8l_sb = pw.tile([128, DC * DH], FP8)
            wq8h_sb = pw.tile([128, E * DC * DH], FP8)
            wq8l_sb = pw.tile([128, E * DC * DH], FP8)
            w28h_sb = pw.tile([128, DC * E * E], FP8)
            biasc_sb = pw.tile([128, 1], F32)
            zeroc_sb = pw.tile([128, 1], F32)
            ident_sb = pw.tile([128, 128], BF16)

            x8 = pkv.tile([128, 2 * DC * S], FP8)      # 16*x [d, (hl, c, t)]
            x8h = x8[:, 0:DC * S]
            x8l = x8[:, DC * S:2 * DC * S]
            k8h = pkv.tile([128, KC * S], FP8)         # 16*K.T  [k, (kc, t)]
            k8l = pkv.tile([128, KC * S], FP8)
            # [t, (tt, e, [V | 1 | U_e])] bf16
            va_sb = pkv.tile([128, TT * E * VW], BF16)

            x8hv = x8h.rearrange("p (c t) -> p c t", c=DC)
            x8lv = x8l.rearrange("p (c t) -> p c t", c=DC)
            k8hv = k8h.rearrange("p (kc t) -> p kc t", kc=KC)
            k8lv = k8l.rearrange("p (kc t) -> p kc t", kc=KC)
            wk8hv = wk8h_sb.rearrange("p (c k) -> p c k", c=DC)
            wk8lv = wk8l_sb.rearrange("p (c k) -> p c k", c=DC)
            wv8hv = wv8h_sb.rearrange("p (c k) -> p c k", c=DC)
            wv8lv = wv8l_sb.rearrange("p (c k) -> p c k", c=DC)
            wq8hv = wq8h_sb.rearrange("p (e c k) -> p e c k", e=E, c=DC)
            wq8lv = wq8l_sb.rearrange("p (e c k) -> p e c k", e=E, c=DC)
            w28hv = w28h_sb.rearrange("p (c q) -> p c q", c=DC)
            vav = va_sb.rearrange("p (t e w) -> p t e w", t=TT, e=E)

            # all input DMAs go on ONE queue in exact first-use order, so
            # arrival order deterministically matches PE consumption order
            # (two queues would split bandwidth and let low-priority weight
            # transfers delay the x8l tiles the V chains stall on)
            nc.sync.dma_start(wv8h_sb[:], wv8h_d[:])

            # ones columns via a Pool-engine memset (a strided DMA here
            # costs ~4us of descriptor generation on the queue)
            nc.gpsimd.memset(vav[:, :, :, DH], 1.0)

            # ====== Phase 1: K, V+router projections; Q prefetch (DR) ======
            with (
                tc.tile_pool(name="pql", bufs=3) as pql,
                tc.tile_pool(name="ps_ql", bufs=1, space="PSUM") as ps_ql,
            ):
              def emit_qproj(st, e, hi_on_dve=True, kcs=None, tiles=None):
                    # q8 = fp8 hi/lo split of Wq_e^T x for s-tile st (DR).
                    # kcs/tiles let the two kc chains be emitted ~4 t-tiles
                    # apart so kc1's matmuls never wait on the qp bank held
                    # by kc0's serial DVE quantize, and the quantize burst
                    # spreads away from the expert boundary
                    if tiles is None:
                        q8h_sb = pql.tile([128, KC * 512], FP8, name="q8h",
                                          tag="qh")
                        q8l_sb = pql.tile([128, KC * 512], FP8, name="q8l",
                                          tag="ql")
                    else:
                        q8h_sb, q8l_sb = tiles
                    nmm = 3 * DP
                    for kc in (range(KC) if kcs is None else kcs):
                        qp = ps_ql.tile([128, 512], F32, name="qp", tag="ql")
                        i = 0
                        for sta, mov in ((wq8hv, x8hv), (wq8hv, x8lv),
                                         (wq8lv, x8hv)):
                            for p in range(DP):
                                nc.tensor.matmul(
                                    qp[:],
                                    sta[:, e, 2 * p:2 * p + 2, kc * 128:(kc + 1) * 128],
                                    mov[:, 2 * p:2 * p + 2, st * 512:(st + 1) * 512],
                                    start=(i == 0), stop=(i == nmm - 1),
                                    perf_mode=DR,
                                )
                                i += 1
                        dh = q8h_sb[:, kc * 512:(kc + 1) * 512]
                        dl = q8l_sb[:, kc * 512:(kc + 1) * 512]
                        # phase 2: quantize fully on DVE — an ACT copy would
                        # land in the expert-boundary window where ACT must
                        # drain the LAG-tail exps (scp-slot WAR stalls PE).
                        # phase 1: hi on ACT — DVE there is busy with K/V/U
                        # lo-splits and the kp-slot WAR would stall K chains.
                        if hi_on_dve:
                            nc.vector.tensor_scalar_mul(dh, qp[:],
                                                        SX / (SX * SW))
                        else:
                            nc.scalar.activation(
                                dh, qp[:], mybir.ActivationFunctionType.Copy,
                                scale=SX / (SX * SW))
                        nc.vector.scalar_tensor_tensor(
                            dl, qp[:], SX / (SX * SW), dh,
                            mybir.AluOpType.mult, mybir.AluOpType.subtract)
                    return q8h_sb, q8l_sb

              units = ([(st, e) for st in range(ST) for e in range(E)]
                       if upto >= 2 else [])
              q8_ready = {}

              with (
                tc.tile_pool(name="ps_kp", bufs=4, space="PSUM") as ps_kp,
                tc.tile_pool(name="ps_vp", bufs=2, space="PSUM") as ps_vp,
                tc.tile_pool(name="ps_up", bufs=1, space="PSUM") as ps_up,
              ):
                nmm = 3 * DP

                # all 16 tiles' U projections share one PSUM bank; one
                # batched quantize per 4 tiles
                def emit_vuk(tt, up_bank):
                    # V tile tt (DR 3-chain): psum = 8192*V
                    vp = ps_vp.tile([128, DH], F32, name="vp")
                    i = 0
                    for sta, mov in ((x8hv, wv8hv), (x8hv, wv8lv), (x8lv, wv8hv)):
                        for p in range(DP):
                            nc.tensor.matmul(
                                vp[:],
                                sta[:, 2 * p:2 * p + 2, tt * 128:(tt + 1) * 128],
                                mov[:, 2 * p:2 * p + 2, :],
                                start=(i == 0), stop=(i == nmm - 1), perf_mode=DR,
                            )
                            i += 1
                    # e=0 from PSUM on ACT, replicas via the idle Pool engine
                    nc.scalar.activation(vav[:, tt, 0, 0:DH], vp[:],
                                         mybir.ActivationFunctionType.Copy,
                                         scale=1.0 / (SX * SW))
                    for e in range(1, E):
                        nc.gpsimd.tensor_copy(vav[:, tt, e, 0:DH],
                                              vav[:, tt, 0, 0:DH])
                    # U tile tt (DR hi-only): psum = 8192*U; 4 tiles share
                    # the bank as one group (start zeroes the whole bank, so
                    # only the first matmul starts and the last one stops)
                    j = tt % 4
                    for p in range(DP):
                        nc.tensor.matmul(
                            up_bank[:, j * E * E:(j + 1) * E * E],
                            x8hv[:, 2 * p:2 * p + 2, tt * 128:(tt + 1) * 128],
                            w28hv[:, 2 * p:2 * p + 2, :],
                            start=(j == 0 and p == 0),
                            stop=(j == 3 and p == DP - 1), perf_mode=DR,
                        )
                    if j == 3:
                        st = tt // 4
                        for jj in range(4):
                            nc.vector.tensor_scalar_mul(
                                vav[:, st * 4 + jj, :, DH + 1:VW],
                                up_bank[:, jj * E * E:(jj + 1) * E * E]
                                .rearrange("p (e q) -> p e q", e=E),
                                1.0 / (SX * SW))
                        # K tiles once this s-tile's 4 x-tiles are in
                        for kc in range(KC):
                            kp = ps_kp.tile([128, 512], F32, name="kp")
                            i = 0
                            for sta, mov in ((wk8hv, x8hv), (wk8hv, x8lv),
                                             (wk8lv, x8hv)):
                                for p in range(DP):
                                    nc.tensor.matmul(
                                        kp[:],
                                        sta[:, 2 * p:2 * p + 2, kc * 128:(kc + 1) * 128],
                                        mov[:, 2 * p:2 * p + 2, st * 512:(st + 1) * 512],
                                        start=(i == 0), stop=(i == nmm - 1),
                                        perf_mode=DR,
                                    )
                                    i += 1
                            dh = k8hv[:, kc, st * 512:(st + 1) * 512]
                            dl = k8lv[:, kc, st * 512:(st + 1) * 512]
                            nc.scalar.activation(dh, kp[:],
                                                 mybir.ActivationFunctionType.Copy,
                                                 scale=SX / (SX * SW))
                            nc.vector.scalar_tensor_tensor(
                                dl, kp[:], SX / (SX * SW), dh,
                                mybir.AluOpType.mult, mybir.AluOpType.subtract)

                # x arrives pre-transposed and pre-split from the host, hi
                # and lo halves in one tensor: one DMA per s-tile (HWDGE
                # descriptor generation is ~0.6us per DMA and serializes
                # across queues, so few+large DMAs in consumption order win);
                # dispatch alternates two queues (650ns+ of sequencer each)
                qs = [nc.scalar, nc.sync]
                qi = [0]

                def dq():
                    q = qs[qi[0] % len(qs)]
                    qi[0] += 1
                    return q

                x8dv = x8_d[:].rearrange("p (g c t) -> p g c t", g=2, c=DC)
                x8v = x8.rearrange("p (g c t) -> p g c t", g=2, c=DC)
                dq().dma_start(x8v[:, 0, :, 0:512], x8dv[:, 0, :, 0:512])
                dq().dma_start(wv8l_sb[:], wv8l_d[:])
                dq().dma_start(x8v[:, 1, :, 0:512], x8dv[:, 1, :, 0:512])
                dq().dma_start(w28h_sb[:], w28h_d[:])
                for st4 in range(ST):
                    lo = st4 * 512
                    if st4 == 0:
                        dq().dma_start(wk8h_sb[:], wk8h_d[:])
                        dq().dma_start(wk8l_sb[:], wk8l_d[:])
                    else:
                        dq().dma_start(x8v[:, :, :, lo:lo + 512],
                                       x8dv[:, :, :, lo:lo + 512])
                    if st4 == 1:
                        # first read in phase 2 (exp bias / router ex)
                        dq().dma_start(biasc_sb[:], biasc_d[:])
                        dq().dma_start(zeroc_sb[:], zeroc_d[:])
                    if st4 == 2:
                        # wq8 is first needed by the st4==2-end Q-prefetch;
                        # it must be EMITTED before that reader (else the
                        # framework orders the DMA after the read as a WAR)
                        # but queued behind st2's x stream
                        dq().dma_start(wq8h_sb[:], wq8h_d[:])
                        dq().dma_start(wq8l_sb[:], wq8l_d[:])
                        dq().dma_start(ident_sb[:], ident_d[:])
                    up_bank = ps_up.tile([128, 4 * E * E], F32, name="up")
                    for tt in range(st4 * 4, st4 * 4 + 4):
                        emit_vuk(tt, up_bank)
                        if st4 == 3 and tt == 13 and units:
                            # prefetch the first units' Q projections late
                            # enough that wq8's 2MB has landed, early enough
                            # that their quantizes hide under remaining vuks;
                            # kc chains split so the qp-bank WAR on the
                            # serial quantize never stalls PE
                            u0_tiles = emit_qproj(*units[0],
                                                  hi_on_dve=False, kcs=(0,))
                        if st4 == 3 and tt == 15 and units:
                            q8_ready[0] = emit_qproj(*units[0],
                                                     hi_on_dve=False,
                                                     kcs=(1,), tiles=u0_tiles)
                            if len(units) > 1:
                                u1_tiles = emit_qproj(*units[1],
                                                      hi_on_dve=False,
                                                      kcs=(0,))
                    if st4 == 3 and len(units) > 1:
                        q8_ready[1] = emit_qproj(*units[1], hi_on_dve=False,
                                                 kcs=(1,), tiles=u1_tiles)

              if upto == 1:
                  nc.sync.dma_start(dbg_k[:], k8h[:])
                  nc.sync.dma_start(dbg_kl[:], k8l[:])
                  nc.sync.dma_start(dbg_v[:], va_sb[:])

              # ========= Phase 2+3: attention, router, combine per s-tile =====
              with (
                  tc.tile_pool(name="pat", bufs=7) as pat,
                  tc.tile_pool(name="pdiag", bufs=6) as pdiag,
                  tc.tile_pool(name="peo", bufs=3) as peo,
                  tc.tile_pool(name="psc3", bufs=2) as psc3,
                  tc.tile_pool(name="pout", bufs=2) as pout,
                  tc.tile_pool(name="ps_sc", bufs=3, space="PSUM") as ps_sc,
                  tc.tile_pool(name="ps_eo", bufs=1, space="PSUM") as ps_eo,
              ):
                  # combine work of tile st-1, emitted piecewise at tile st's
                  # expert boundaries so it never bunches up on DVE right
                  # when the next expert's PSUM banks need their drains
                  pending_combine = []
                  # eoB drains (ACT) deferred past the boundary's exp burst
                  pending_drain = []
                  last_drainB = [None]

                  for st in (range(ST) if upto >= 2 else ()):
                      # [128, (e, ch, 256)] bf16 drained expert outputs
                      eo_buf = peo.tile([128, E * NCH * 256], BF16, name="eo_buf")
                      rr_t = psc3.tile([128, E * NCH], F32, name="rr_t", tag="rr")
                      lacc = psc3.tile([128, NCH * E], F32, name="lacc", tag="lacc")
                      for e in range(E):
                          q8h_sb, q8l_sb = q8_ready.pop(st * E + e)
                          q8hvv = q8h_sb.rearrange("p (kc s) -> p kc s", kc=KC)
                          q8lvv = q8l_sb.rearrange("p (kc s) -> p kc s", kc=KC)
                          uidx = st * E + e
                          # eo/rowsum/logit accumulators: chunk c of each
                          # pair-tile lives at col 512c (its own PSUM bank)
                          eoA = ps_eo.tile([128, 1024], F32, name="eoA", tag="eoA")
                          eoB = ps_eo.tile([128, 1024], F32, name="eoB", tag="eoB")
                          eop = [eoA, eoB]
                          NB = TT
                          ats = [None] * NB
                          # software pipeline: sc/exp of tile k runs 5 tiles
                          # ahead of the eo consumers so PE never waits on ACT
                          LAG = 5
                          for it in range(NB + LAG):
                              if it < NB:
                                  t = it
                                  scp = ps_sc.tile([128, 512], F32, name="scp")
                                  j = 0
                                  for sta, mov in ((k8hv, q8hvv), (k8hv, q8lvv),
                                                   (k8lv, q8hvv)):
                                      nc.tensor.matmul(
                                          scp[:],
                                          sta[:, :, t * 128:(t + 1) * 128],
                                          mov[:, :, :],
                                          start=(j == 0), stop=(j == 2),
                                          perf_mode=DR,
                                      )
                                      j += 1
                                  at = pat.tile([128, 512], BF16, name="at")
                                  nc.scalar.activation(at[:], scp[:],
                                                       mybir.ActivationFunctionType.Exp,
                                                       scale=1.0 / (SX * SX * SCALE),
                                                       bias=biasc_sb[:])
                                  ats[t] = at
                              if it == 2 and pending_drain:
                                  pending_drain.pop(0)()
                              if it == NB - 5 and uidx + 2 < len(units):
                                  # pipeline: project Q two units ahead so its
                                  # fp8 quantize never gates a unit's scores
                                  qtiles = emit_qproj(*units[uidx + 2],
                                                      kcs=(0,))
                              if it == NB - 1 and uidx + 2 < len(units):
                                  q8_ready[uidx + 2] = emit_qproj(
                                      *units[uidx + 2], kcs=(1,), tiles=qtiles)
                              if it < LAG:
                                  continue
                              t = it - LAG
                              at = ats[t]
                              first, last = (t == 0), (t == TT - 1)
                              for ch in range(NCH):
                                  sl = at[:, ch * 128:(ch + 1) * 128]
                                  # one ap=261 matmul accumulates eo +
                                  # rowsum + router logits for this chunk
                                  nc.tensor.matmul(
                                      eop[ch // 2][:, (ch % 2) * 512:
                                                   (ch % 2) * 512 + VW],
                                      sl,
                                      va_sb[:, (t * E + e) * VW:
                                            (t * E + e + 1) * VW],
                                      start=first, stop=last,
                                  )
                          # ---- drain this expert: reciprocals and logit
                          # accumulation first (smallest ops, release the
                          # aug columns), then the eoA copy on DVE; the eoB
                          # copy (ACT) is DEFERRED into the next expert's
                          # batch loop so it never delays the tail exps that
                          # free scp slots at the boundary ----
                          eoAv = eoA.rearrange("p (c w) -> p c w", c=2)
                          eoBv = eoB.rearrange("p (c w) -> p c w", c=2)
                          nc.vector.reciprocal(rr_t[:, e * NCH:e * NCH + 2],
                                               eoAv[:, :, DH])
                          nc.vector.reciprocal(rr_t[:, e * NCH + 2:e * NCH + 4],
                                               eoBv[:, :, DH])
                          for ch in range(NCH):
                              src = (eoAv if ch < 2 else eoBv)[:, ch % 2,
                                                               DH + 1:VW]
                              dst = lacc[:, ch * E:(ch + 1) * E]
                              rr_s = rr_t[:, e * NCH + ch:e * NCH + ch + 1]
                              if e == 0:
                                  nc.vector.tensor_scalar_mul(dst, src, rr_s)
                              else:
                                  nc.vector.scalar_tensor_tensor(
                                      dst, src, rr_s, dst,
                                      mybir.AluOpType.mult, mybir.AluOpType.add,
                                  )
                          ebv = eo_buf.rearrange("p (q w) -> p q w", q=E * NCH)
                          if st == ST - 1 and e == E - 1:
                              # per-chunk copies: the first diag-combine
                              # chain only needs chunk 0, so don't make it
                              # wait for a full 512-wide drain
                              nc.vector.tensor_copy(
                                  ebv[:, e * NCH:e * NCH + 1, :],
                                  eoAv[:, 0:1, 0:DH])
                              nc.vector.tensor_copy(
                                  ebv[:, e * NCH + 1:e * NCH + 2, :],
                                  eoAv[:, 1:2, 0:DH])
                          else:
                              nc.vector.tensor_copy(
                                  ebv[:, e * NCH:e * NCH + 2, :],
                                  eoAv[:, :, 0:DH])

                          def drainB(ebv=ebv, eoBv=eoBv, e=e):
                              nc.scalar.activation(
                                  ebv[:, e * NCH + 2:e * NCH + 4, :],
                                  eoBv[:, :, 0:DH],
                                  mybir.ActivationFunctionType.Copy)

                          if st == ST - 1 and e == E - 1:
                              # deferred into the tail: emitted after the
                              # first router-exp so ACT doesn't delay the
                              # diag pipeline start (its data is only read
                              # by the ch2/ch3 chains, ~2us later)
                              last_drainB[0] = drainB
                          else:
                              pending_drain.append(drainB)
                          if pending_combine:
                              pending_combine.pop(0)()
                      # ---- router softmax + combine ----
                      ex = psc3.tile([128, NCH * E], F32, name="ex", tag="ex")
                      sumx = psc3.tile([128, NCH], F32, name="sumx", tag="sumx")
                      rw = psc3.tile([128, NCH], F32, name="rw", tag="rw")
                      wn = psc3.tile([128, NCH * E], F32, name="wn", tag="wn")
                      rrv = rr_t.rearrange("p (e c) -> p c e", e=E)
                      ob = pout.tile([128, NCH * DH], BF16, name="ob")
                      last = st == ST - 1

                      def wn_chunk(ch):
                          # router logits are O(0.1), so exp without
                          # max-subtraction; wn = softmax(lacc) * rrec
                          nc.scalar.activation(ex[:, ch * E:(ch + 1) * E],
                                               lacc[:, ch * E:(ch + 1) * E],
                                               mybir.ActivationFunctionType.Exp,
                                               bias=zeroc_sb[:])
                          nc.vector.tensor_reduce(
                              sumx[:, ch:ch + 1],
                              ex[:, ch * E:(ch + 1) * E],
                              mybir.AxisListType.X, mybir.AluOpType.add)
                          nc.vector.reciprocal(rw[:, ch:ch + 1],
                                               sumx[:, ch:ch + 1])
                          nc.vector.tensor_scalar_mul(wn[:, ch * E:(ch + 1) * E],
                                                      ex[:, ch * E:(ch + 1) * E],
                                                      rw[:, ch:ch + 1])
                          nc.vector.tensor_tensor(
                              wn[:, ch * E:(ch + 1) * E],
                              wn[:, ch * E:(ch + 1) * E],
                              rrv[:, ch, :],
                              mybir.AluOpType.mult)

                      if last:
                          # tail combine on the now-idle PE, pipelined per
                          # chunk: wn chain (DVE/ACT) -> diag(w)-matmuls
                          # accumulate w_e*eo_e in PSUM -> drain -> store
                          obps = [ps_eo.tile([128, 1024], F32, name="obA",
                                             tag="eoA"),
                                  ps_eo.tile([128, 1024], F32, name="obB",
                                             tag="eoB")]
                          for ch in range(NCH):
                              if ch == 0:
                                  wn_chunk(0)
                                  last_drainB[0]()
                              dst = obps[ch // 2][:, (ch % 2) * 512:
                                                  (ch % 2) * 512 + DH]
                              for e in range(E):
                                  dg = pdiag.tile([128, 128], BF16, name="dg")
                                  nc.vector.tensor_scalar_mul(
                                      dg[:], ident_sb[:],
                                      wn[:, ch * E + e:ch * E + e + 1])
                                  nc.tensor.matmul(
                                      dst,
                                      dg[:],
                                      eo_buf[:, (e * NCH + ch) * 256:
                                             (e * NCH + ch) * 256 + 256],
                                      start=(e == 0), stop=(e == E - 1),
                                  )
                                  if e == 0 and ch + 1 < NCH:
                                      # next chunk's router weights computed
                                      # under this chunk's diag matmuls so
                                      # the PE never waits on the wn chain
                                      wn_chunk(ch + 1)
                              och = ob[:, ch * DH:(ch + 1) * DH]
                              if ch % 2 == 0:
                                  nc.vector.tensor_copy(och, dst)
                              else:
                                  nc.scalar.activation(
                                      och, dst,
                                      mybir.ActivationFunctionType.Copy)
                              lo = st * 512 + ch * 128
                              nc.sync.dma_start(out_d[lo:lo + 128, :], och)
                      else:
                          # batched wn (fewer instructions; latency hidden
                          # under the next tile's PE work)
                          nc.scalar.activation(ex[:], lacc[:],
                                               mybir.ActivationFunctionType.Exp,
                                               bias=zeroc_sb[:])
                          nc.vector.tensor_reduce(
                              sumx[:],
                              ex[:].rearrange("p (c e) -> p c e", c=NCH),
                              mybir.AxisListType.X, mybir.AluOpType.add)
                          nc.vector.reciprocal(rw[:], sumx[:])
                          for ch in range(NCH):
                              nc.vector.tensor_scalar_mul(
                                  wn[:, ch * E:(ch + 1) * E],
                                  ex[:, ch * E:(ch + 1) * E],
                                  rw[:, ch:ch + 1])
                          nc.vector.tensor_tensor(
                              wn[:].rearrange("p (c e) -> p c e", c=NCH),
                              wn[:].rearrange("p (c e) -> p c e", c=NCH),
                              rrv[:],
                              mybir.AluOpType.mult)
                          # bf16 SBUF combine on DVE (TensorScalarPtr is
                          # DVE-only); emitted one expert-layer at a time at
                          # the NEXT tile's expert boundaries
                          def combine_layer(ce, ob=ob, eo_buf=eo_buf,
                                            wn=wn, st=st):
                              for ch in range(NCH):
                                  src = eo_buf[:, (ce * NCH + ch) * 256:
                                               (ce * NCH + ch) * 256 + 256]
                                  w_s = wn[:, ch * E + ce:ch * E + ce + 1]
                                  dst = ob[:, ch * DH:(ch + 1) * DH]
                                  if ce == 0:
                                      nc.vector.tensor_scalar_mul(dst, src, w_s)
                                  else:
                                      nc.vector.scalar_tensor_tensor(
                                          dst, src, w_s, dst,
                                          mybir.AluOpType.mult,
                                          mybir.AluOpType.add,
                                      )
                              if ce == E - 1:
                                  nc.sync.dma_start(
                                      out_d[st * 512:(st + 1) * 512, :]
                                      .rearrange("(c p) k -> p c k", c=NCH),
                                      ob[:].rearrange("p (c k) -> p c k",
                                                      c=NCH))
                          for ce in range(E):
                              pending_combine.append(
                                  lambda ce=ce: combine_layer(ce))

    nc.compile()
    return nc


def _get_nc():
    global _cached
    if _cached is None:
        _cached = _build()
    return _cached


FP8NP = ml_dtypes.float8_e4m3


def _q8pair(a, s):
    hi = (a * s).astype(FP8NP)
    lo = (a * s - hi.astype(np.float32)).astype(FP8NP)
    assert np.isfinite(hi.astype(np.float32)).all()
    return hi, lo


def _host_prep(x, Wq, Wk, Wv, Wr):
    def chunked(w):  # [D, N] -> [128, DC*N] with layout [p, (c, n)]
        n = w.shape[1]
        return np.ascontiguousarray(
            w.reshape(DC, 128, n).transpose(1, 0, 2).reshape(128, DC * n))

    # host-side transpose + fp8 hi/lo split of x (device [p,(hl,c,t)])
    x8s = []
    for b in range(B):
        xT = np.ascontiguousarray(
            x[b].T.reshape(DC, 128, S).transpose(1, 0, 2).reshape(128, DC * S))
        hi = (xT * SX).astype(FP8NP)
        lo = (xT * SX - hi.astype(np.float32)).astype(FP8NP)
        x8s.append(np.ascontiguousarray(np.concatenate([hi, lo], axis=1)))

    in_maps = []
    for c in range(NCORES):
        b, h = divmod(c, H)
        wq_h = Wq[h].reshape(E, DC, 128, DH).transpose(2, 0, 1, 3).reshape(
            128, E * DC * DH)
        wv_h = Wv[:, h * DH:(h + 1) * DH]
        # W2[:, e, e'] = Wv_h @ Wr_h[e-block]  -> [D, E, E]
        w2 = np.stack([wv_h @ Wr[h, e * DH:(e + 1) * DH, :] for e in range(E)],
                      axis=1).reshape(D, E * E)
        wk8h, wk8l = _q8pair(chunked(Wk[:, h * DH:(h + 1) * DH]), SW)
        wv8h, wv8l = _q8pair(chunked(wv_h), SW)
        wq8h, wq8l = _q8pair(wq_h, SW)
        w28h, _ = _q8pair(chunked(w2), SW)
        in_maps.append({
            "x8": x8s[b],
            "wk8h": wk8h, "wk8l": wk8l,
            "wv8h": wv8h, "wv8l": wv8l,
            "wq8h": np.ascontiguousarray(wq8h), "wq8l": np.ascontiguousarray(wq8l),
            "w28h": w28h,
            "biasc": np.full((128, 1), -LN4, dtype=np.float32),
            "zeroc": np.zeros((128, 1), dtype=np.float32),
            "ident": np.eye(128, dtype=ml_dtypes.bfloat16),
        })
    return in_maps


def kernel(x, Wq, Wk, Wv, Wr):
    global _last_in_maps
    x = np.asarray(x, dtype=np.float32)
    Wq = np.asarray(Wq, dtype=np.float32)
    Wk = np.asarray(Wk, dtype=np.float32)
    Wv = np.asarray(Wv, dtype=np.float32)
    Wr = np.asarray(Wr, dtype=np.float32)

    nc = _get_nc()
    in_maps = _host_prep(x, Wq, Wk, Wv, Wr)
    _last_in_maps = in_maps
    res = bass_utils.run_bass_kernel_spmd(nc, in_maps, core_ids=list(range(NCORES)))

    out = np.empty((B, S, H, DH), dtype=np.float32)
    for c in range(NCORES):
        b, h = divmod(c, H)
        out[b, :, h, :] = np.asarray(res.results[c]["out"]).astype(np.float32)
    return out


# revision 71
# speedup vs baseline: 1.0047x; 1.0047x over previous
"""MoE multi-head attention Trainium2 kernel (v4, fused router columns).

Problem: x:[B=2,S=2048,D=1024], Wq:[H=4,E=4,D,DH=256], Wk/Wv:[D,D], Wr:[H,E*DH,E]
  K/V = per-head projections of x; Q per (head, expert); full softmax attention
  per (b,h,e); router softmax over experts from concat of expert outputs;
  router-weighted combine -> out [B,S,H,DH].

Sharding: 8 cores = B*H (2 batches x 4 heads); all E=4 experts core-local.

Large GEMMs run as fp8e4 DoubleRow matmuls (2 contraction tiles per pass,
0.5 cyc/row) with hi/lo error compensation (3-chain ah*bh + ah*bl + al*bh).
Attention weights `at` stay bf16: an fp8 `at` (or any dropped chain) was
measured at 2-4e-2 final error vs the 2e-2 gate, so eo runs bf16 at the
bf16 roofline.

v4 change vs v3: the router/rowsum path no longer uses 1024 separate tiny
matmuls (the PE *sequencer* costs ~75ns/matmul — those "free" ap=5 matmuls
were 77us of SEQ time). Instead each expert's V tile carries 5 fused
columns [ones | U_e(4)] (U = Wv@Wr precomputed on host), so the eo
accumulation chain computes eo, the softmax rowsum, AND the per-expert
router logits in a single ap=261 matmul per (t,chunk).
PSUM (8 banks): scores 3x1 + eoA/eoB 2x2 (chunk c of a pair-tile at col
512c, own bank each) + a dedicated Q-projection bank.

Scheduling notes (each worth ~0.5-3us of PE idle, found via TimelineSim
traces):
 - one large DMA per s-tile of x, two queues alternating, in exact
   consumption order (HWDGE descriptor gen ~0.6us serializes transfers
   across queues; dispatch costs ~0.7-1.3us of sequencer per DMA);
   a DMA must be EMITTED before its first reader or the tile framework
   orders it as WAR after the read.
 - sc/exp per single t-tile, eo consumers LAG=5 tiles behind.
 - Q-projection for unit u+2 emitted inside unit u's batch loop; its fp8
   quantize runs fully on DVE in phase 2 (an ACT copy would land in the
   boundary window where ACT must drain the LAG-tail exps that free scp
   slots) but on ACT in phase 1 (there DVE is the congested engine).
 - each expert's eoB drain is deferred into the NEXT expert's batch loop
   (same reason); eoA drain + reciprocals + logit-accumulate stay at the
   boundary, smallest ops first.
 - the previous tile's 16-op DVE combine is emitted one expert-layer at a
   time at the next tile's expert boundaries.
 - last tile: combine runs on the then-idle PE as diag(w_e) @ eo_e PSUM
   accumulation chains (identity row-scaled by DVE), pipelined per chunk
   behind a per-chunk router-softmax chain, with per-chunk stores.
"""
import sys

sys.path.insert(0, "/opt/trn_rl_repo")

import math

import numpy as np
import ml_dtypes

import concourse.bass as bass
import concourse.mybir as mybir
import concourse.tile as tile
from concourse import bacc, bass_utils

B, S, D = 2, 2048, 1024
H, E, DH = 4, 4, 256
SCALE = math.sqrt(DH)
NCORES = B * H

DC = D // 128      # 8 contraction chunks over D
DP = DC // 2       # 4 DoubleRow chunk-pairs over D
KC = DH // 128     # 2 chunks over head dim
ST = S // 512      # 4 tiles of 512 queries
TT = S // 128      # 16 tiles of 128 tokens
NCH = 4            # 128-query chunks per s-tile
VW = DH + 1 + E    # per-(tt,e) va block: [V(256) | ones | U_e(4)]
LN4 = math.log(4.0)

SX = 16.0          # fp8 scale for x, K, Q values (|v| ~ N(0,1))
SW = 512.0         # fp8 scale for weights (|w| ~ N(0, 1/1024))

F32 = mybir.dt.float32
BF16 = mybir.dt.bfloat16
FP8 = mybir.dt.float8e4
DR = mybir.MatmulPerfMode.DoubleRow

_cached = None
_last_in_maps = None


def _build(upto=3):
    nc = bacc.Bacc("TRN2", target_bir_lowering=False, debug=False)

    # hi and lo halves in ONE tensor: one DMA per s-tile (HWDGE descriptor
    # generation is ~0.6us per DMA and serializes across queues)
    x8_d = nc.dram_tensor("x8", [128, 2 * DC * S], FP8, kind="ExternalInput")
    wk8h_d = nc.dram_tensor("wk8h", [128, DC * DH], FP8, kind="ExternalInput")
    wk8l_d = nc.dram_tensor("wk8l", [128, DC * DH], FP8, kind="ExternalInput")
    wv8h_d = nc.dram_tensor("wv8h", [128, DC * DH], FP8, kind="ExternalInput")
    wv8l_d = nc.dram_tensor("wv8l", [128, DC * DH], FP8, kind="ExternalInput")
    wq8h_d = nc.dram_tensor("wq8h", [128, E * DC * DH], FP8, kind="ExternalInput")
    wq8l_d = nc.dram_tensor("wq8l", [128, E * DC * DH], FP8, kind="ExternalInput")
    w28h_d = nc.dram_tensor("w28h", [128, DC * E * E], FP8, kind="ExternalInput")
    biasc_d = nc.dram_tensor("biasc", [128, 1], F32, kind="ExternalInput")
    zeroc_d = nc.dram_tensor("zeroc", [128, 1], F32, kind="ExternalInput")
    ident_d = nc.dram_tensor("ident", [128, 128], BF16, kind="ExternalInput")
    out_d = nc.dram_tensor("out", [S, DH], BF16, kind="ExternalOutput")
    if upto == 1:
        dbg_k = nc.dram_tensor("dbg_k", [128, KC * S], FP8, kind="ExternalOutput")
        dbg_kl = nc.dram_tensor("dbg_kl", [128, KC * S], FP8, kind="ExternalOutput")
        dbg_v = nc.dram_tensor("dbg_v", [128, TT * E * VW], BF16, kind="ExternalOutput")

    with tile.TileContext(nc) as tc:
        with (
            tc.tile_pool(name="pw", bufs=1) as pw,
            tc.tile_pool(name="pkv", bufs=1) as pkv,
        ):
            # ---- resident weights/constants ----
            wk8h_sb = pw.tile([128, DC * DH], FP8)
            wk8l_sb = pw.tile([128, DC * DH], FP8)
            wv8h_sb = pw.tile([128, DC * DH], FP8)
            wv8l_sb = pw.tile([128, DC * DH], FP8)
            wq8h_sb = pw.tile([128, E * DC * DH], FP8)
            wq8l_sb = pw.tile([128, E * DC * DH], FP8)
            w28h_sb = pw.tile([128, DC * E * E], FP8)
            biasc_sb = pw.tile([128, 1], F32)
            zeroc_sb = pw.tile([128, 1], F32)
            ident_sb = pw.tile([128, 128], BF16)

            x8 = pkv.tile([128, 2 * DC * S], FP8)      # 16*x [d, (hl, c, t)]
            x8h = x8[:, 0:DC * S]
            x8l = x8[:, DC * S:2 * DC * S]
            k8h = pkv.tile([128, KC * S], FP8)         # 16*K.T  [k, (kc, t)]
            k8l = pkv.tile([128, KC * S], FP8)
            # [t, (tt, e, [V | 1 | U_e])] bf16
            va_sb = pkv.tile([128, TT * E * VW], BF16)

            x8hv = x8h.rearrange("p (c t) -> p c t", c=DC)
            x8lv = x8l.rearrange("p (c t) -> p c t", c=DC)
            k8hv = k8h.rearrange("p (kc t) -> p kc t", kc=KC)
            k8lv = k8l.rearrange("p (kc t) -> p kc t", kc=KC)
            wk8hv = wk8h_sb.rearrange("p (c k) -> p c k", c=DC)
            wk8lv = wk8l_sb.rearrange("p (c k) -> p c k", c=DC)
            wv8hv = wv8h_sb.rearrange("p (c k) -> p c k", c=DC)
            wv8lv = wv8l_sb.rearrange("p (c k) -> p c k", c=DC)
            wq8hv = wq8h_sb.rearrange("p (e c k) -> p e c k", e=E, c=DC)
            wq8lv = wq8l_sb.rearrange("p (e c k) -> p e c k", e=E, c=DC)
            w28hv = w28h_sb.rearrange("p (c q) -> p c q", c=DC)
            vav = va_sb.rearrange("p (t e w) -> p t e w", t=TT, e=E)

            # all input DMAs go on ONE queue in exact first-use order, so
            # arrival order deterministically matches PE consumption order
            # (two queues would split bandwidth and let low-priority weight
            # transfers delay the x8l tiles the V chains stall on)
            nc.sync.dma_start(wv8h_sb[:], wv8h_d[:])

            # ones columns via a Pool-engine memset (a strided DMA here
            # costs ~4us of descriptor generation on the queue)
            nc.gpsimd.memset(vav[:, :, :, DH], 1.0)

            # ====== Phase 1: K, V+router projections; Q prefetch (DR) ======
            with (
                tc.tile_pool(name="pql", bufs=3) as pql,
                tc.tile_pool(name="ps_ql", bufs=1, space="PSUM") as ps_ql,
            ):
              def emit_qproj(st, e, hi_on_dve=True, kcs=None, tiles=None):
                    # q8 = fp8 hi/lo split of Wq_e^T x for s-tile st (DR).
                    # kcs/tiles let the two kc chains be emitted ~4 t-tiles
                    # apart so kc1's matmuls never wait on the qp bank held
                    # by kc0's serial DVE quantize, and the quantize burst
                    # spreads away from the expert boundary
                    if tiles is None:
                        q8h_sb = pql.tile([128, KC * 512], FP8, name="q8h",
                                          tag="qh")
                        q8l_sb = pql.tile([128, KC * 512], FP8, name="q8l",
                                          tag="ql")
                    else:
                        q8h_sb, q8l_sb = tiles
                    nmm = 3 * DP
                    for kc in (range(KC) if kcs is None else kcs):
                        qp = ps_ql.tile([128, 512], F32, name="qp", tag="ql")
                        i = 0
                        for sta, mov in ((wq8hv, x8hv), (wq8hv, x8lv),
                                         (wq8lv, x8hv)):
                            for p in range(DP):
                                nc.tensor.matmul(
                                    qp[:],
                                    sta[:, e, 2 * p:2 * p + 2, kc * 128:(kc + 1) * 128],
                                    mov[:, 2 * p:2 * p + 2, st * 512:(st + 1) * 512],
                                    start=(i == 0), stop=(i == nmm - 1),
                                    perf_mode=DR,
                                )
                                i += 1
                        dh = q8h_sb[:, kc * 512:(kc + 1) * 512]
                        dl = q8l_sb[:, kc * 512:(kc + 1) * 512]
                        # phase 2: quantize fully on DVE — an ACT copy would
                        # land in the expert-boundary window where ACT must
                        # drain the LAG-tail exps (scp-slot WAR stalls PE).
                        # phase 1: hi on ACT — DVE there is busy with K/V/U
                        # lo-splits and the kp-slot WAR would stall K chains.
                        if hi_on_dve:
                            nc.vector.tensor_scalar_mul(dh, qp[:],
                                                        SX / (SX * SW))
                        else:
                            nc.scalar.activation(
                                dh, qp[:], mybir.ActivationFunctionType.Copy,
                                scale=SX / (SX * SW))
                        nc.vector.scalar_tensor_tensor(
                            dl, qp[:], SX / (SX * SW), dh,
                            mybir.AluOpType.mult, mybir.AluOpType.subtract)
                    return q8h_sb, q8l_sb

              units = ([(st, e) for st in range(ST) for e in range(E)]
                       if upto >= 2 else [])
              q8_ready = {}

              with (
                tc.tile_pool(name="ps_kp", bufs=4, space="PSUM") as ps_kp,
                tc.tile_pool(name="ps_vp", bufs=2, space="PSUM") as ps_vp,
                tc.tile_pool(name="ps_up", bufs=1, space="PSUM") as ps_up,
              ):
                nmm = 3 * DP

                # all 16 tiles' U projections share one PSUM bank; one
                # batched quantize per 4 tiles
                def emit_vuk(tt, up_bank):
                    # V tile tt (DR 3-chain): psum = 8192*V
                    vp = ps_vp.tile([128, DH], F32, name="vp")
                    i = 0
                    for sta, mov in ((x8hv, wv8hv), (x8hv, wv8lv), (x8lv, wv8hv)):
                        for p in range(DP):
                            nc.tensor.matmul(
                                vp[:],
                                sta[:, 2 * p:2 * p + 2, tt * 128:(tt + 1) * 128],
                                mov[:, 2 * p:2 * p + 2, :],
                                start=(i == 0), stop=(i == nmm - 1), perf_mode=DR,
                            )
                            i += 1
                    # e=0 from PSUM on ACT, replicas via the idle Pool engine
                    nc.scalar.activation(vav[:, tt, 0, 0:DH], vp[:],
                                         mybir.ActivationFunctionType.Copy,
                                         scale=1.0 / (SX * SW))
                    for e in range(1, E):
                        nc.gpsimd.tensor_copy(vav[:, tt, e, 0:DH],
                                              vav[:, tt, 0, 0:DH])
                    # U tile tt (DR hi-only): psum = 8192*U; 4 tiles share
                    # the bank as one group (start zeroes the whole bank, so
                    # only the first matmul starts and the last one stops)
                    j = tt % 4
                    for p in range(DP):
                        nc.tensor.matmul(
                            up_bank[:, j * E * E:(j + 1) * E * E],
                            x8hv[:, 2 * p:2 * p + 2, tt * 128:(tt + 1) * 128],
                            w28hv[:, 2 * p:2 * p + 2, :],
                            start=(j == 0 and p == 0),
                            stop=(j == 3 and p == DP - 1), perf_mode=DR,
                        )
                    if j == 3:
                        st = tt // 4
                        for jj in range(4):
                            nc.vector.tensor_scalar_mul(
                                vav[:, st * 4 + jj, :, DH + 1:VW],
                                up_bank[:, jj * E * E:(jj + 1) * E * E]
                                .rearrange("p (e q) -> p e q", e=E),
                                1.0 / (SX * SW))
                        # K tiles once this s-tile's 4 x-tiles are in
                        for kc in range(KC):
                            kp = ps_kp.tile([128, 512], F32, name="kp")
                            i = 0
                            for sta, mov in ((wk8hv, x8hv), (wk8hv, x8lv),
                                             (wk8lv, x8hv)):
                                for p in range(DP):
                                    nc.tensor.matmul(
                                        kp[:],
                                        sta[:, 2 * p:2 * p + 2, kc * 128:(kc + 1) * 128],
                                        mov[:, 2 * p:2 * p + 2, st * 512:(st + 1) * 512],
                                        start=(i == 0), stop=(i == nmm - 1),
                                        perf_mode=DR,
                                    )
                                    i += 1
                            dh = k8hv[:, kc, st * 512:(st + 1) * 512]
                            dl = k8lv[:, kc, st * 512:(st + 1) * 512]
                            nc.scalar.activation(dh, kp[:],
                                                 mybir.ActivationFunctionType.Copy,
                                                 scale=SX / (SX * SW))
                            nc.vector.scalar_tensor_tensor(
                                dl, kp[:], SX / (SX * SW), dh,
                                mybir.AluOpType.mult, mybir.AluOpType.subtract)

                # x arrives pre-transposed and pre-split from the host, hi
                # and lo halves in one tensor: one DMA per s-tile (HWDGE
                # descriptor generation is ~0.6us per DMA and serializes
                # across queues, so few+large DMAs in consumption order win);
                # dispatch alternates two queues (650ns+ of sequencer each)
                qs = [nc.scalar, nc.sync]
                qi = [0]

                def dq():
                    q = qs[qi[0] % len(qs)]
                    qi[0] += 1
                    return q

                x8dv = x8_d[:].rearrange("p (g c t) -> p g c t", g=2, c=DC)
                x8v = x8.rearrange("p (g c t) -> p g c t", g=2, c=DC)
                dq().dma_start(x8v[:, 0, :, 0:512], x8dv[:, 0, :, 0:512])
                dq().dma_start(wv8l_sb[:], wv8l_d[:])
                dq().dma_start(x8v[:, 1, :, 0:512], x8dv[:, 1, :, 0:512])
                dq().dma_start(w28h_sb[:], w28h_d[:])
                for st4 in range(ST):
                    lo = st4 * 512
                    if st4 == 0:
                        dq().dma_start(wk8h_sb[:], wk8h_d[:])
                        dq().dma_start(wk8l_sb[:], wk8l_d[:])
                    else:
                        dq().dma_start(x8v[:, :, :, lo:lo + 512],
                                       x8dv[:, :, :, lo:lo + 512])
                    if st4 == 1:
                        # first read in phase 2 (exp bias / router ex)
                        dq().dma_start(biasc_sb[:], biasc_d[:])
                        dq().dma_start(zeroc_sb[:], zeroc_d[:])
                    if st4 == 2:
                        # wq8 is first needed by the st4==2-end Q-prefetch;
                        # it must be EMITTED before that reader (else the
                        # framework orders the DMA after the read as a WAR)
                        # but queued behind st2's x stream
                        dq().dma_start(wq8h_sb[:], wq8h_d[:])
                        dq().dma_start(wq8l_sb[:], wq8l_d[:])
                        dq().dma_start(ident_sb[:], ident_d[:])
                    up_bank = ps_up.tile([128, 4 * E * E], F32, name="up")
                    for tt in range(st4 * 4, st4 * 4 + 4):
                        emit_vuk(tt, up_bank)
                        if st4 == 3 and tt == 13 and units:
                            # prefetch the first units' Q projections late
                            # enough that wq8's 2MB has landed, early enough
                            # that their quantizes hide under remaining vuks
                            q8_ready[0] = emit_qproj(*units[0],
                                                     hi_on_dve=False)
                    if st4 == 3 and len(units) > 1:
                        q8_ready[1] = emit_qproj(*units[1], hi_on_dve=False)

              if upto == 1:
                  nc.sync.dma_start(dbg_k[:], k8h[:])
                  nc.sync.dma_start(dbg_kl[:], k8l[:])
                  nc.sync.dma_start(dbg_v[:], va_sb[:])

              # ========= Phase 2+3: attention, router, combine per s-tile =====
              with (
                  tc.tile_pool(name="pat", bufs=7) as pat,
                  tc.tile_pool(name="pdiag", bufs=6) as pdiag,
                  tc.tile_pool(name="peo", bufs=3) as peo,
                  tc.tile_pool(name="psc3", bufs=2) as psc3,
                  tc.tile_pool(name="pout", bufs=2) as pout,
                  tc.tile_pool(name="ps_sc", bufs=3, space="PSUM") as ps_sc,
                  tc.tile_pool(name="ps_eo", bufs=1, space="PSUM") as ps_eo,
              ):
                  # combine work of tile st-1, emitted piecewise at tile st's
                  # expert boundaries so it never bunches up on DVE right
                  # when the next expert's PSUM banks need their drains
                  pending_combine = []
                  # eoB drains (ACT) deferred past the boundary's exp burst
                  pending_drain = []
                  last_drainB = [None]

                  for st in (range(ST) if upto >= 2 else ()):
                      # [128, (e, ch, 256)] bf16 drained expert outputs
                      eo_buf = peo.tile([128, E * NCH * 256], BF16, name="eo_buf")
                      rr_t = psc3.tile([128, E * NCH], F32, name="rr_t", tag="rr")
                      lacc = psc3.tile([128, NCH * E], F32, name="lacc", tag="lacc")
                      for e in range(E):
                          q8h_sb, q8l_sb = q8_ready.pop(st * E + e)
                          q8hvv = q8h_sb.rearrange("p (kc s) -> p kc s", kc=KC)
                          q8lvv = q8l_sb.rearrange("p (kc s) -> p kc s", kc=KC)
                          uidx = st * E + e
                          # eo/rowsum/logit accumulators: chunk c of each
                          # pair-tile lives at col 512c (its own PSUM bank)
                          eoA = ps_eo.tile([128, 1024], F32, name="eoA", tag="eoA")
                          eoB = ps_eo.tile([128, 1024], F32, name="eoB", tag="eoB")
                          eop = [eoA, eoB]
                          NB = TT
                          ats = [None] * NB
                          # software pipeline: sc/exp of tile k runs 5 tiles
                          # ahead of the eo consumers so PE never waits on ACT
                          LAG = 5
                          for it in range(NB + LAG):
                              if it < NB:
                                  t = it
                                  scp = ps_sc.tile([128, 512], F32, name="scp")
                                  j = 0
                                  for sta, mov in ((k8hv, q8hvv), (k8hv, q8lvv),
                                                   (k8lv, q8hvv)):
                                      nc.tensor.matmul(
                                          scp[:],
                                          sta[:, :, t * 128:(t + 1) * 128],
                                          mov[:, :, :],
                                          start=(j == 0), stop=(j == 2),
                                          perf_mode=DR,
                                      )
                                      j += 1
                                  at = pat.tile([128, 512], BF16, name="at")
                                  nc.scalar.activation(at[:], scp[:],
                                                       mybir.ActivationFunctionType.Exp,
                                                       scale=1.0 / (SX * SX * SCALE),
                                                       bias=biasc_sb[:])
                                  ats[t] = at
                              if it == 2 and pending_drain:
                                  pending_drain.pop(0)()
                              if it == NB - 8 and uidx + 2 < len(units):
                                  # pipeline: project Q two units ahead so its
                                  # fp8 quantize never gates a unit's scores;
                                  # both kc chains (and their serial DVE
                                  # quantizes) sit fully clear of the
                                  # boundary's drain window
                                  qtiles = emit_qproj(*units[uidx + 2],
                                                      kcs=(0,))
                              if it == NB - 4 and uidx + 2 < len(units):
                                  q8_ready[uidx + 2] = emit_qproj(
                                      *units[uidx + 2], kcs=(1,), tiles=qtiles)
                              if it < LAG:
                                  continue
                              t = it - LAG
                              at = ats[t]
                              first, last = (t == 0), (t == TT - 1)
                              for ch in range(NCH):
                                  sl = at[:, ch * 128:(ch + 1) * 128]
                                  # one ap=261 matmul accumulates eo +
                                  # rowsum + router logits for this chunk
                                  nc.tensor.matmul(
                                      eop[ch // 2][:, (ch % 2) * 512:
                                                   (ch % 2) * 512 + VW],
                                      sl,
                                      va_sb[:, (t * E + e) * VW:
                                            (t * E + e + 1) * VW],
                                      start=first, stop=last,
                                  )
                          # ---- drain this expert: reciprocals and logit
                          # accumulation first (smallest ops, release the
                          # aug columns), then the eoA copy on DVE; the eoB
                          # copy (ACT) is DEFERRED into the next expert's
                          # batch loop so it never delays the tail exps that
                          # free scp slots at the boundary ----
                          eoAv = eoA.rearrange("p (c w) -> p c w", c=2)
                          eoBv = eoB.rearrange("p (c w) -> p c w", c=2)
                          nc.vector.reciprocal(rr_t[:, e * NCH:e * NCH + 2],
                                               eoAv[:, :, DH])
                          nc.vector.reciprocal(rr_t[:, e * NCH + 2:e * NCH + 4],
                                               eoBv[:, :, DH])
                          for ch in range(NCH):
                              src = (eoAv if ch < 2 else eoBv)[:, ch % 2,
                                                               DH + 1:VW]
                              dst = lacc[:, ch * E:(ch + 1) * E]
                              rr_s = rr_t[:, e * NCH + ch:e * NCH + ch + 1]
                              if e == 0:
                                  nc.vector.tensor_scalar_mul(dst, src, rr_s)
                              else:
                                  nc.vector.scalar_tensor_tensor(
                                      dst, src, rr_s, dst,
                                      mybir.AluOpType.mult, mybir.AluOpType.add,
                                  )
                          ebv = eo_buf.rearrange("p (q w) -> p q w", q=E * NCH)
                          if st == ST - 1 and e == E - 1:
                              # per-chunk copies: the first diag-combine
                              # chain only needs chunk 0, so don't make it
                              # wait for a full 512-wide drain
                              nc.vector.tensor_copy(
                                  ebv[:, e * NCH:e * NCH + 1, :],
                                  eoAv[:, 0:1, 0:DH])
                              nc.vector.tensor_copy(
                                  ebv[:, e * NCH + 1:e * NCH + 2, :],
                                  eoAv[:, 1:2, 0:DH])
                          else:
                              nc.vector.tensor_copy(
                                  ebv[:, e * NCH:e * NCH + 2, :],
                                  eoAv[:, :, 0:DH])

                          def drainB(ebv=ebv, eoBv=eoBv, e=e):
                              nc.scalar.activation(
                                  ebv[:, e * NCH + 2:e * NCH + 4, :],
                                  eoBv[:, :, 0:DH],
                                  mybir.ActivationFunctionType.Copy)

                          if st == ST - 1 and e == E - 1:
                              # deferred into the tail: emitted after the
                              # first router-exp so ACT doesn't delay the
                              # diag pipeline start (its data is only read
                              # by the ch2/ch3 chains, ~2us later)
                              last_drainB[0] = drainB
                          else:
                              pending_drain.append(drainB)
                          if pending_combine:
                              pending_combine.pop(0)()
                      # ---- router softmax + combine ----
                      ex = psc3.tile([128, NCH * E], F32, name="ex", tag="ex")
                      sumx = psc3.tile([128, NCH], F32, name="sumx", tag="sumx")
                      rw = psc3.tile([128, NCH], F32, name="rw", tag="rw")
                      wn = psc3.tile([128, NCH * E], F32, name="wn", tag="wn")
                      rrv = rr_t.rearrange("p (e c) -> p c e", e=E)
                      ob = pout.tile([128, NCH * DH], BF16, name="ob")
                      last = st == ST - 1

                      def wn_chunk(ch):
                          # router logits are O(0.1), so exp without
                          # max-subtraction; wn = softmax(lacc) * rrec
                          nc.scalar.activation(ex[:, ch * E:(ch + 1) * E],
                                               lacc[:, ch * E:(ch + 1) * E],
                                               mybir.ActivationFunctionType.Exp,
                                               bias=zeroc_sb[:])
                          nc.vector.tensor_reduce(
                              sumx[:, ch:ch + 1],
                              ex[:, ch * E:(ch + 1) * E],
                              mybir.AxisListType.X, mybir.AluOpType.add)
                          nc.vector.reciprocal(rw[:, ch:ch + 1],
                                               sumx[:, ch:ch + 1])
                          nc.vector.tensor_scalar_mul(wn[:, ch * E:(ch + 1) * E],
                                                      ex[:, ch * E:(ch + 1) * E],
                                                      rw[:, ch:ch + 1])
                          nc.vector.tensor_tensor(
                              wn[:, ch * E:(ch + 1) * E],
                              wn[:, ch * E:(ch + 1) * E],
                              rrv[:, ch, :],
                              mybir.AluOpType.mult)

                      if last:
                          # tail combine on the now-idle PE, pipelined per
                          # chunk: wn chain (DVE/ACT) -> diag(w)-matmuls
                          # accumulate w_e*eo_e in PSUM -> drain -> store
                          obps = [ps_eo.tile([128, 1024], F32, name="obA",
                                             tag="eoA"),
                                  ps_eo.tile([128, 1024], F32, name="obB",
                                             tag="eoB")]
                          for ch in range(NCH):
                              if ch == 0:
                                  wn_chunk(0)
                                  last_drainB[0]()
                              dst = obps[ch // 2][:, (ch % 2) * 512:
                                                  (ch % 2) * 512 + DH]
                              for e in range(E):
                                  dg = pdiag.tile([128, 128], BF16, name="dg")
                                  nc.vector.tensor_scalar_mul(
                                      dg[:], ident_sb[:],
                                      wn[:, ch * E + e:ch * E + e + 1])
                                  nc.tensor.matmul(
                                      dst,
                                      dg[:],
                                      eo_buf[:, (e * NCH + ch) * 256:
                                             (e * NCH + ch) * 256 + 256],
                                      start=(e == 0), stop=(e == E - 1),
                                  )
                                  if e == 0 and ch + 1 < NCH:
                                      # next chunk's router weights computed
                                      # under this chunk's diag matmuls so
                                      # the PE never waits on the wn chain
                                      wn_chunk(ch + 1)
                              och = ob[:, ch * DH:(ch + 1) * DH]
                              if ch % 2 == 0:
                                  nc.vector.tensor_copy(och, dst)
                              else:
                                  nc.scalar.activation(
                                      och, dst,
                                      mybir.ActivationFunctionType.Copy)
                              lo = st * 512 + ch * 128
                              nc.sync.dma_start(out_d[lo:lo + 128, :], och)
                      else:
                          # batched wn (fewer instructions; latency hidden
                          # under the next tile's PE work)
                          nc.scalar.activation(ex[:], lacc[:],
                                               mybir.ActivationFunctionType.Exp,
                                               bias=zeroc_sb[:])
                          nc.vector.tensor_reduce(
                              sumx[:],
                              ex[:].rearrange("p (c e) -> p c e", c=NCH),
                              mybir.AxisListType.X, mybir.AluOpType.add)
                          nc.vector.reciprocal(rw[:], sumx[:])
                          for ch in range(NCH):
                              nc.vector.tensor_scalar_mul(
                                  wn[:, ch * E:(ch + 1) * E],
                                  ex[:, ch * E:(ch + 1) * E],
                                  rw[:, ch:ch + 1])
                          nc.vector.tensor_tensor(
                              wn[:].rearrange("p (c e) -> p c e", c=NCH),
                              wn[:].rearrange("p (c e) -> p c e", c=NCH),
                              rrv[:],
                              mybir.AluOpType.mult)
                          # bf16 SBUF combine on DVE (TensorScalarPtr is
                          # DVE-only); emitted one expert-layer at a time at
                          # the NEXT tile's expert boundaries
                          def combine_layer(ce, ob=ob, eo_buf=eo_buf,
                                            wn=wn, st=st):
                              for ch in range(NCH):
                                  src = eo_buf[:, (ce * NCH + ch) * 256:
                                               (ce * NCH + ch) * 256 + 256]
                                  w_s = wn[:, ch * E + ce:ch * E + ce + 1]
                                  dst = ob[:, ch * DH:(ch + 1) * DH]
                                  if ce == 0:
                                      nc.vector.tensor_scalar_mul(dst, src, w_s)
                                  else:
                                      nc.vector.scalar_tensor_tensor(
                                          dst, src, w_s, dst,
                                          mybir.AluOpType.mult,
                                          mybir.AluOpType.add,
                                      )
                              if ce == E - 1:
                                  nc.sync.dma_start(
                                      out_d[st * 512:(st + 1) * 512, :]
                                      .rearrange("(c p) k -> p c k", c=NCH),
                                      ob[:].rearrange("p (c k) -> p c k",
                                                      c=NCH))
                          for ce in range(E):
                              pending_combine.append(
                                  lambda ce=ce: combine_layer(ce))

    nc.compile()
    return nc


def _get_nc():
    global _cached
    if _cached is None:
        _cached = _build()
    return _cached


FP8NP = ml_dtypes.float8_e4m3


def _q8pair(a, s):
    hi = (a * s).astype(FP8NP)
    lo = (a * s - hi.astype(np.float32)).astype(FP8NP)
    assert np.isfinite(hi.astype(np.float32)).all()
    return hi, lo


def _host_prep(x, Wq, Wk, Wv, Wr):
    def chunked(w):  # [D, N] -> [128, DC*N] with layout [p, (c, n)]
        n = w.shape[1]
        return np.ascontiguousarray(
            w.reshape(DC, 128, n).transpose(1, 0, 2).reshape(128, DC * n))

    # host-side transpose + fp8 hi/lo split of x (device [p,(hl,c,t)])
    x8s = []
    for b in range(B):
        xT = np.ascontiguousarray(
            x[b].T.reshape(DC, 128, S).transpose(1, 0, 2).reshape(128, DC * S))
        hi = (xT * SX).astype(FP8NP)
        lo = (xT * SX - hi.astype(np.float32)).astype(FP8NP)
        x8s.append(np.ascontiguousarray(np.concatenate([hi, lo], axis=1)))

    in_maps = []
    for c in range(NCORES):
        b, h = divmod(c, H)
        wq_h = Wq[h].reshape(E, DC, 128, DH).transpose(2, 0, 1, 3).reshape(
            128, E * DC * DH)
        wv_h = Wv[:, h * DH:(h + 1) * DH]
        # W2[:, e, e'] = Wv_h @ Wr_h[e-block]  -> [D, E, E]
        w2 = np.stack([wv_h @ Wr[h, e * DH:(e + 1) * DH, :] for e in range(E)],
                      axis=1).reshape(D, E * E)
        wk8h, wk8l = _q8pair(chunked(Wk[:, h * DH:(h + 1) * DH]), SW)
        wv8h, wv8l = _q8pair(chunked(wv_h), SW)
        wq8h, wq8l = _q8pair(wq_h, SW)
        w28h, _ = _q8pair(chunked(w2), SW)
        in_maps.append({
            "x8": x8s[b],
            "wk8h": wk8h, "wk8l": wk8l,
            "wv8h": wv8h, "wv8l": wv8l,
            "wq8h": np.ascontiguousarray(wq8h), "wq8l": np.ascontiguousarray(wq8l),
            "w28h": w28h,
            "biasc": np.full((128, 1), -LN4, dtype=np.float32),
            "zeroc": np.zeros((128, 1), dtype=np.float32),
            "ident": np.eye(128, dtype=ml_dtypes.bfloat16),
        })
    return in_maps


def kernel(x, Wq, Wk, Wv, Wr):
    global _last_in_maps
    x = np.asarray(x, dtype=np.float32)
    Wq = np.asarray(Wq, dtype=np.float32)
    Wk = np.asarray(Wk, dtype=np.float32)
    Wv = np.asarray(Wv, dtype=np.float32)
    Wr = np.asarray(Wr, dtype=np.float32)

    nc = _get_nc()
    in_maps = _host_prep(x, Wq, Wk, Wv, Wr)
    _last_in_maps = in_maps
    res = bass_utils.run_bass_kernel_spmd(nc, in_maps, core_ids=list(range(NCORES)))

    out = np.empty((B, S, H, DH), dtype=np.float32)
    for c in range(NCORES):
        b, h = divmod(c, H)
        out[b, :, h, :] = np.asarray(res.results[c]["out"]).astype(np.float32)
    return out
